# revision 1
# baseline (speedup 1.0000x reference)
"""MoE feed-forward (8 experts, hard argmin routing) on 8 TRN2 NeuronCores.

Strategy
--------
Host (numpy): rms_norm + argmin routing (0.13% of FLOPs), then a dispatch
plan: tokens sorted by expert, padded to 128-token tiles, packed into a
UNIFORM per-core structure -- every core runs the same static program of
K expert-segments with identical tile counts; only the DATA (which expert's
weights, which tokens) differs per core.  Weights/activations are cast to
bf16 on host (halves the dominant HBM traffic; fp32 PSUM accumulation).

Device (Bass/Tile, SPMD x8): per segment, stream the segment's expert
weights into SBUF, then for each token group (<=512 tokens) run
weight-stationary matmuls: up-proj (K=1024 contraction) -> swiglu
(ACT Silu + DVE mul) -> down-proj (K=2048 contraction), writing yT back to
DRAM in fp32.

Host: scatter y back to token order and add the skip connection.
"""

import json
import math

import ml_dtypes
import numpy as np

N_EXPERTS = 8
DIM = 1024
HID = 2048
N_CORES = 8
P = 128
EPS = 1e-6

BF16 = ml_dtypes.bfloat16


# ----------------------------------------------------------------------------
# BIR fixup: walrus in this container accepts at most ONE sync-wait per
# instruction.  Split instructions with k>1 waits into (k-1) pure-wait
# EventSemaphore instructions on the same engine immediately before.
# ----------------------------------------------------------------------------
def _split_multiwait_json(bir_bytes: bytes) -> bytes:
    m = json.loads(bir_bytes)
    ctr = 0
    for func in m["functions"]:
        for bb in func["blocks"]:
            out = []
            for inst in bb["instructions"]:
                si = inst.get("sync_info")
                waits = (si or {}).get("on_wait") or []
                if len(waits) > 1:
                    for w in waits[:-1]:
                        ctr += 1
                        out.append({
                            "debug": inst.get("debug", 0),
                            "engine": inst["engine"],
                            "ins": [],
                            "outs": [],
                            "name": f"waitfix_{ctr}",
                            "opcode": "EventSemaphore",
                            "sync_info": {"on_update": [], "on_wait": [w]},
                        })
                    si["on_wait"] = [waits[-1]]
                out.append(inst)
            bb["instructions"] = out
    return json.dumps(m).encode()


def _patch_bass_json(nc):
    orig = nc.to_json_bytes

    def patched():
        return _split_multiwait_json(orig())

    nc.to_json_bytes = patched


# ----------------------------------------------------------------------------
# Host-side routing (replicates the reference numerics in fp32)
# ----------------------------------------------------------------------------
def _route(x, scale, centroids):
    xf = x.reshape(-1, DIM).astype(np.float32)
    ms = np.mean(xf * xf, axis=-1, keepdims=True)
    s = scale.astype(np.float32) / np.sqrt(ms + EPS)
    xn = xf * s
    nx = np.sum(xn * xn, axis=-1)[:, None]
    ny = np.sum(centroids * centroids, axis=-1)[None, :]
    d2 = nx + ny - 2.0 * (xn @ centroids.T)
    ids = np.argmin(d2, axis=-1).astype(np.int32)
    return xn, ids


# ----------------------------------------------------------------------------
# Dispatch planner: uniform per-core segment structure
# ----------------------------------------------------------------------------
def _compositions(total, k):
    """Descending compositions of `total` into exactly k positive parts."""
    if k == 1:
        yield (total,)
        return
    for first in range(total - k + 1, 0, -1):
        for rest in _compositions(total - first, k - 1):
            if rest[0] <= first:
                yield (first,) + rest


def _try_pack(tiles_per_expert, comp, n_cores):
    """Greedy: biggest expert chunk -> biggest remaining slot.
    Returns slot assignment {(core, seg): expert or None} or None."""
    slots = []  # (size, core, seg)
    for c in range(n_cores):
        for j, sz in enumerate(comp):
            slots.append([sz, c, j])
    slots.sort(key=lambda s: -s[0])
    remaining = [(t, e) for e, t in enumerate(tiles_per_expert) if t > 0]
    remaining.sort(key=lambda te: -te[0])
    assign = {}
    used = [False] * len(slots)
    chunks = {}  # (core, seg) -> n real tiles
    for t, e in remaining:
        r = t
        while r > 0:
            best = None
            for i, (sz, c, j) in enumerate(slots):
                if used[i]:
                    continue
                if best is None:
                    best = i
                if sz <= r:
                    best = i
                    break
            if best is None:
                return None
            sz, c, j = slots[best]
            used[best] = True
            take = min(r, sz)
            assign[(c, j)] = e
            chunks[(c, j)] = take
            r -= take
    return assign, chunks


def _plan(ids):
    tok_by_e = [np.where(ids == e)[0] for e in range(N_EXPERTS)]
    tiles_e = [(len(t) + P - 1) // P for t in tok_by_e]
    nt = sum(tiles_e)
    tpc = max(1, (nt + N_CORES - 1) // N_CORES)
    for extra in range(0, 3):
        t = tpc + extra
        for k in range(1, 5):
            for comp in _compositions(t, k):
                res = _try_pack(tiles_e, comp, N_CORES)
                if res is not None:
                    assign, chunks = res
                    return comp, assign, chunks, tok_by_e
    raise RuntimeError("dispatch packing failed")


# ----------------------------------------------------------------------------
# Device program
# ----------------------------------------------------------------------------
def _build_program(comp):
    import concourse.bass as bass
    import concourse.mybir as mybir
    import concourse.tile as tile

    f32 = mybir.dt.float32
    bf16 = mybir.dt.bfloat16
    Silu = mybir.ActivationFunctionType.Silu

    K = len(comp)
    T = sum(comp) * P  # token slots per core

    nc = bass.Bass("TRN2", debug=False)
    xnt_in = nc.dram_tensor("xnt", [P, 8, T], bf16, kind="ExternalInput").ap()
    up_in = nc.dram_tensor("up", [K, 8, P, 2 * HID], bf16, kind="ExternalInput").ap()
    down_in = nc.dram_tensor("down", [K, 16, P, DIM], bf16, kind="ExternalInput").ap()
    yt_out = nc.dram_tensor("yt", [8, P, T], f32, kind="ExternalOutput").ap()

    with tile.TileContext(nc) as tc:
        with (
            tc.tile_pool(name="upw", bufs=9) as up_pool,
            tc.tile_pool(name="dnw", bufs=17) as dn_pool,
            tc.tile_pool(name="xn", bufs=2) as xn_pool,
            tc.tile_pool(name="act", bufs=2) as act_pool,
            tc.tile_pool(name="sg", bufs=4) as sg_pool,
            tc.tile_pool(name="yc", bufs=8) as yc_pool,
            tc.tile_pool(name="psA", bufs=4, space="PSUM") as psA,
            tc.tile_pool(name="psB", bufs=4, space="PSUM") as psB,
        ):
            col = 0
            for s in range(K):
                up_sb = []
                for ko in range(8):
                    t = up_pool.tile([P, 2 * HID], bf16, tag="upw")
                    nc.sync.dma_start(t[:], up_in[s, ko])
                    up_sb.append(t)
                dn_sb = []
                for kh in range(16):
                    t = dn_pool.tile([P, DIM], bf16, tag="dnw")
                    nc.sync.dma_start(t[:], down_in[s, kh])
                    dn_sb.append(t)

                # token groups of <=4 tiles
                rem = comp[s]
                while rem > 0:
                    g = min(4, rem)
                    rem -= g
                    gn = g * P
                    xn_t = xn_pool.tile([P, 8, gn], bf16, tag="xn")
                    nc.sync.dma_start(xn_t[:], xnt_in[:, :, col : col + gn])
                    act_t = act_pool.tile([P, 16, gn], bf16, tag="act")
                    for j in range(16):
                        pa = psA.tile([P, gn], f32, tag="psA")
                        pg = psA.tile([P, gn], f32, tag="psA")
                        for ko in range(8):
                            nc.tensor.matmul(
                                pa[:],
                                up_sb[ko][:, j * P : (j + 1) * P],
                                xn_t[:, ko, :],
                                start=(ko == 0),
                                stop=(ko == 7),
                            )
                        for ko in range(8):
                            nc.tensor.matmul(
                                pg[:],
                                up_sb[ko][:, (16 + j) * P : (17 + j) * P],
                                xn_t[:, ko, :],
                                start=(ko == 0),
                                stop=(ko == 7),
                            )
                        sg = sg_pool.tile([P, gn], f32, tag="sg")
                        nc.scalar.activation(sg[:], pg[:], Silu)
                        nc.vector.tensor_mul(act_t[:, j, :], pa[:], sg[:])
                    for do in range(8):
                        pd = psB.tile([P, gn], f32, tag="psB")
                        for kh in range(16):
                            nc.tensor.matmul(
                                pd[:],
                                dn_sb[kh][:, do * P : (do + 1) * P],
                                act_t[:, kh, :],
                                start=(kh == 0),
                                stop=(kh == 15),
                            )
                        yc = yc_pool.tile([P, gn], f32, tag="yc")
                        nc.vector.tensor_copy(yc[:], pd[:])
                        nc.sync.dma_start(yt_out[do, :, col : col + gn], yc[:])
                    col += gn

    _patch_bass_json(nc)
    return nc


# ----------------------------------------------------------------------------
# Entry point
# ----------------------------------------------------------------------------
def _run(inputs, trace=False, tmpdir=None):
    from concourse.bass_utils import run_bass_kernel_spmd

    x = np.asarray(inputs["x"])
    scale = np.asarray(inputs["scale"])
    centroids = np.asarray(inputs["centroids"])
    up_w = np.asarray(inputs["up_w"])
    down_w = np.asarray(inputs["down_w"])

    B, S, D = x.shape
    ntok = B * S
    xf32 = x.reshape(ntok, D).astype(np.float32)

    xn, ids = _route(x, scale, centroids)
    comp, assign, chunks, tok_by_e = _plan(ids)
    K = len(comp)
    T = sum(comp) * P

    # ---- pack per-core inputs ----
    xnT = np.ascontiguousarray(xn.T)  # [DIM, ntok] f32
    # consumed-token cursor per expert
    cursor = [0] * N_EXPERTS
    core_cols_tok = [np.zeros(T, dtype=np.int64) for _ in range(N_CORES)]
    core_cols_valid = [np.zeros(T, dtype=bool) for _ in range(N_CORES)]
    up_bf = up_w.astype(BF16)  # [E, DIM, 2H]
    down_bf = down_w.astype(BF16)  # [E, HID, DIM]
    in_maps = []
    seg_tok_ranges = []  # per core: list of (colstart, ncols)
    for c in range(N_CORES):
        up_pack = np.zeros((K, 8, P, 2 * HID), dtype=BF16)
        down_pack = np.zeros((K, 16, P, DIM), dtype=BF16)
        col = 0
        for j, sz in enumerate(comp):
            e = assign.get((c, j))
            ncols = sz * P
            if e is not None:
                up_pack[j] = up_bf[e].reshape(8, P, 2 * HID)
                down_pack[j] = down_bf[e].reshape(16, P, DIM)
                toks = tok_by_e[e]
                take = min(chunks[(c, j)] * P, len(toks) - cursor[e])
                take = max(take, 0)
                if take:
                    sel = toks[cursor[e] : cursor[e] + take]
                    cursor[e] += take
                    core_cols_tok[c][col : col + take] = sel
                    core_cols_valid[c][col : col + take] = True
            col += ncols
        # gather activations (pad columns point at token 0 = harmless garbage)
        xnt_cols = xnT[:, core_cols_tok[c]].astype(BF16)  # [DIM, T]
        xnt_pack = np.ascontiguousarray(
            xnt_cols.reshape(8, P, T).transpose(1, 0, 2)
        )  # [P, 8, T]
        in_maps.append({"xnt": xnt_pack, "up": up_pack, "down": down_pack})

    for e in range(N_EXPERTS):
        assert cursor[e] == len(tok_by_e[e]), "dispatch did not cover all tokens"

    nc = _build_program(comp)
    kwargs = {}
    if trace:
        kwargs = dict(trace=True, tmpdir=tmpdir)
    res = run_bass_kernel_spmd(nc, in_maps, core_ids=list(range(N_CORES)), **kwargs)

    # ---- scatter + skip ----
    out = xf32.copy()
    for c in range(N_CORES):
        yt = res.results[c]["yt"].reshape(8 * P, T)  # [DIM, T]
        valid = core_cols_valid[c]
        toks = core_cols_tok[c][valid]
        out[toks] = xf32[toks] + yt[:, valid].T
    return out.reshape(B, S, D).astype(x.dtype), res


def kernel(**inputs) -> np.ndarray:
    out, _ = _run(inputs)
    return out
